# revision 19
# baseline (speedup 1.0000x reference)
"""Sharded AtomTransformer (sparse local attention, +/-64 window) on 8 NeuronCores.

Sequence-parallel: each core owns 256 rows and computes on a 768-row slab
(6 tiles of 128, [own-256, own+512), zero-padded at the sequence ends).
Per block i: dense projections on kv_tiles, attention + transition only on a
shrinking pyramid of out_tiles ({6,4,2}); validity shrinks 64 rows/side/block,
so all 256 owned rows are exact after block 2 with zero communication.

Attention is banded: a 128-query tile attends to two 128-key chunks at
[-64,+64) and [+64,+192); softmax is factored as exp(S)*exp(pair_bias) with
exp(pair_bias) carrying the band/validity mask (exact zeros).  The pair-bias
table is computed on device from a host-gathered bf16 band of LayerNorm'd z
(6 tiles x [128q, 256k, 16c] per core) - far smaller than shipping z slabs.
Matmul inputs are cast to bf16 (outputs accumulate f32).
"""

import numpy as np
import ml_dtypes

N = 2048
C = 128
CP = 16
H = 8
D = 16
NB = 3
WIN = 64
NC = 8
SLAB_T = 6
SLAB = SLAB_T * 128
LHALO = 256
KV_TILES = [(0, 6), (0, 6), (1, 5)]
OUT_TILES = [(0, 6), (1, 5), (2, 4)]
BF16 = ml_dtypes.bfloat16

_compiled = None


def _device_fn(a, s, znb, gtile, mask_pad, a_gw, a_gb, a_bw, wq, bq, wk, wv,
               wbp, beta, wg, wo, wog, bog, t_gw, t_gb, t_bw, w1, w2, w3,
               wsg, bsg):
    """One core's slab.
    a, s: [SLAB, C] f32;  znb: [SLAB_T, 128, 256, CP] int8 (LN'd z band,
    scaled by 32; the 1/32 is folded into wbp host-side);
    gtile: [SLAB_T] i32 global tile index (or -1 if out of range);
    mask_pad: [N+256] f32 0/1 key mask padded 128 each side;
    weights per block lists; wbp [NB, CP, H], beta [NB, H].
    """
    import jax
    import jax.numpy as jnp
    f32 = jnp.float32
    bf = jnp.bfloat16

    # band/validity mask on device: [SLAB_T, 128q, 256k] {0, -inf}
    jj = jnp.arange(256)
    qq = jnp.arange(128)
    band_ok = jnp.abs(qq[:, None] - (jj[None, :] - 64)) <= WIN      # [128,256]
    k_glob = 128 * gtile[:, None] - 64 + jj[None, :]                # [T,256]
    key_ok = (mask_pad[k_glob + 128] > 0) & (k_glob >= 0) & (k_glob < N) \
        & (gtile[:, None] >= 0)
    pbmask = jnp.where(band_ok[None] & key_ok[:, None, :], 0.0,
                       -jnp.inf).astype(f32)

    def ln(x):
        m = x.mean(-1, keepdims=True)
        v = ((x - m) ** 2).mean(-1, keepdims=True)
        return (x - m) * jax.lax.rsqrt(v + 1e-5)

    sig = jax.nn.sigmoid
    inv_sqrt_d = np.float32(1.0 / np.sqrt(D))
    sn = ln(s)
    snb = sn.astype(bf)
    sb = s.astype(bf)

    # pair-bias tables for all blocks: [NB, SLAB_T, 128q, 256k, H]
    # (exp is applied per block on the pyramid's out_tiles only)
    pb = jnp.einsum('tqkc,ich->itqkh', znb.astype(f32), wbp.astype(f32),
                    precision='highest') + beta[:, None, None, None, :]

    for i in range(NB):
        kv_lo, kv_hi = KV_TILES[i]
        out_lo, out_hi = OUT_TILES[i]
        an_ln = ln(a)
        gate = sig(snb @ a_gw[i] + a_gb[i])
        anb = gate * an_ln + snb @ a_bw[i]
        anbb = anb.astype(bf)
        q = (anbb @ wq[i] + bq[i]).astype(bf)          # [SLAB, hd]
        k = anbb @ wk[i]                                # [SLAB, hd] bf16
        v = anbb @ wv[i]
        g = sig(anbb @ wg[i])
        og = sig(sb @ wog[i] + bog[i])
        sg_ = sig(sb @ wsg[i] + bsg[i])
        # padded keys/values: col = slab_row + 64
        kp = jnp.pad(k, ((64, 64), (0, 0)))
        vp = jnp.pad(v, ((64, 64), (0, 0)))
        nt = out_hi - out_lo
        qs = q[128 * out_lo:128 * out_hi].reshape(nt, 128, H, D)
        ks = jnp.stack([jax.lax.dynamic_slice_in_dim(kp, 128 * t, 256)
                        for t in range(out_lo, out_hi)])   # [nt, 256, hd]
        vs = jnp.stack([jax.lax.dynamic_slice_in_dim(vp, 128 * t, 256)
                        for t in range(out_lo, out_hi)])
        ks = ks.reshape(nt, 256, H, D)
        vs = vs.reshape(nt, 256, H, D)
        sc = jnp.einsum('tqhd,tkhd->thqk', qs, ks,
                        preferred_element_type=f32) * inv_sqrt_d
        pbe = jnp.exp(pb[i, out_lo:out_hi]
                      + pbmask[out_lo:out_hi, :, :, None])
        u = jnp.exp(sc) * jnp.transpose(pbe, (0, 3, 1, 2))    # [nt,h,q,k]
        den = u.sum(-1) + 1e-20
        o = jnp.einsum('thqk,tkhd->tqhd', u.astype(bf), vs,
                       preferred_element_type=f32) / jnp.transpose(
            den, (0, 2, 1))[:, :, :, None]
        go = (o.reshape(nt * 128, C) * g[128 * out_lo:128 * out_hi]).astype(bf)
        attn_out = og[128 * out_lo:128 * out_hi] * (go @ wo[i])
        # transition
        tgate = sig(snb[128 * out_lo:128 * out_hi] @ t_gw[i] + t_gb[i])
        tnb = (tgate * an_ln[128 * out_lo:128 * out_hi]
               + snb[128 * out_lo:128 * out_hi] @ t_bw[i]).astype(bf)
        h1 = tnb @ w1[i]
        h2 = tnb @ w2[i]
        hid = (jax.nn.silu(h1) * h2).astype(bf)
        tr = sg_[128 * out_lo:128 * out_hi] * (hid @ w3[i])
        a = a.at[128 * out_lo:128 * out_hi].set(attn_out + tr)
    return a[256:512]


def _get_compiled():
    global _compiled
    if _compiled is not None:
        return _compiled
    import jax
    devs = jax.devices()[:NC]
    _compiled = jax.pmap(_device_fn, devices=devs,
                         in_axes=(0, 0, 0, 0, None) + (0,) * 21)
    return _compiled


def _slab(core, x):
    lo = 256 * core - LHALO
    out = np.zeros((SLAB,) + x.shape[1:], x.dtype)
    g0, g1 = max(lo, 0), min(lo + SLAB, N)
    if g1 > g0:
        out[g0 - lo:g1 - lo] = x[g0:g1]
    return out


_K_CLIP = None


def _k_clip():
    global _K_CLIP
    if _K_CLIP is None:
        gt = np.arange(16)
        k_idx = 128 * gt[:, None] - 64 + np.arange(256)[None, :]   # [16,256]
        _K_CLIP = np.clip(k_idx, 0, N - 1)
    return _K_CLIP


def kernel(atom_single_repr, atom_single_proj, atom_pair_repr,
           a_gw, a_gb, a_bw, a_ss, wq, bq, wk, wv, z_s, z_b, wb,
           wg, wo, wog, bog, t_gw, t_gb, t_bw, t_ss, w1, w2, w3, wsg, bsg,
           mask):
    f32 = np.float32
    a_full = np.asarray(atom_single_repr, f32)[0]
    s_full = np.asarray(atom_single_proj, f32)[0]
    z_full = np.asarray(atom_pair_repr, f32)[0]
    m_full = np.asarray(mask).reshape(-1).astype(bool)

    k_clip = _k_clip()

    # global z band [16, 128, 256, CP] -> LN -> int8 (x32; |ln| < 3.88)
    znb_g = np.empty((16, 128, 256, CP), np.int8)
    for t in range(16):
        zb = z_full[128 * t:128 * t + 128][:, k_clip[t]]
        m = zb.mean(-1, keepdims=True)
        vv = ((zb - m) ** 2).mean(-1, keepdims=True)
        zn = (zb - m) / np.sqrt(vv + 1e-5)
        znb_g[t] = np.clip(np.rint(zn * 32.0), -127, 127).astype(np.int8)

    a_sh = np.stack([_slab(c, a_full) for c in range(NC)])
    s_sh = np.stack([_slab(c, s_full) for c in range(NC)])
    # per-core band tiles: global tiles 2c-2 .. 2c+3 (zero-padded OOR)
    znb_sh = np.zeros((NC, SLAB_T, 128, 256, CP), np.int8)
    gtile = np.full((NC, SLAB_T), -1, np.int32)
    for c in range(NC):
        for t in range(SLAB_T):
            gt = 2 * c + t - 2
            if 0 <= gt < 16:
                znb_sh[c, t] = znb_g[gt]
                gtile[c, t] = gt
    mask_pad = np.zeros(N + 256, f32)
    mask_pad[128:128 + N] = m_full.astype(f32)

    ass = np.asarray(a_ss, f32)
    tss = np.asarray(t_ss, f32)
    wbp = (np.asarray(z_s, f32)[:, :, None] * np.asarray(wb, f32)) / 32.0
    beta = np.einsum('ic,ich->ih', np.asarray(z_b, f32), np.asarray(wb, f32))

    def b16(x):
        return np.asarray(x, f32).astype(BF16)

    wts = (b16(ass[:, :, None] * np.asarray(a_gw, f32)),
           np.asarray(a_gb, f32),
           b16(ass[:, :, None] * np.asarray(a_bw, f32)),
           b16(wq), np.asarray(bq, f32).astype(BF16), b16(wk), b16(wv),
           b16(wbp), np.asarray(beta, f32),
           b16(wg), b16(wo), b16(wog), np.asarray(bog, f32),
           b16(tss[:, :, None] * np.asarray(t_gw, f32)),
           np.asarray(t_gb, f32),
           b16(tss[:, :, None] * np.asarray(t_bw, f32)),
           b16(w1), b16(w2), b16(w3), b16(wsg), np.asarray(bsg, f32))
    wts = _cached_weights(wts)

    fn = _get_compiled()
    out = fn(a_sh, s_sh, znb_sh, gtile, mask_pad, *wts)
    return np.asarray(out, f32).reshape(N, C)[None]


_WCACHE = {}


def _cached_weights(wts):
    """Keep the (per-call identical) weight arrays resident on all devices;
    re-upload only when their contents change."""
    import hashlib
    import jax
    h = hashlib.md5()
    for w in wts:
        h.update(w.tobytes())
    key = h.hexdigest()
    if key not in _WCACHE:
        devs = jax.devices()[:NC]
        _WCACHE.clear()
        _WCACHE[key] = [jax.device_put_replicated(w, devs) for w in wts]
    return _WCACHE[key]
